# revision 3
# baseline (speedup 1.0000x reference)
"""GQA attention block (RoPE + causal attention + out-proj) on 8 TRN2 cores.

Problem: nn_AdvancedAttn (B=2, S=2048, DIM=2048, H=16 q-heads, KVH=4 kv-heads,
DH=128), fp32 in/out.

Sharding: core (b, g) for b in {0,1}, g in {0..3} handles batch b and kv-head
group g (4 query heads + 1 kv head).  Wq/Wk/Wv are split along the head dim,
Wo along its input dim; the 4 partial Wo outputs per batch are summed on host
(the all-reduce of tensor parallelism).

Device kernel (per core):
  - The Q/K/V and output projections run as fp8 DoubleRow matmuls (0.5
    cycles/row) on a 3-product hi/lo expansion: each operand is split into
    two e4m3 codes (hi = e4m3(t), lo = e4m3(t - hi)) and the product
    (hi+lo)x(hi+lo) is computed dropping the lo*lo term.  This costs 0.75
    cycles/row instead of fp32r's 1.0 with ~tf32 accuracy (the dropped term
    is ~2^-10 relative).  Chunk pairs ride the two DoubleRow k-tiles, so no
    operand duplication is needed.  x / W codes are prepared on host; the
    attention-output codes are produced on device during softmax
    normalization.
  - Scores / AV / rowsum stay fp32r with Q^T/K^T produced in [dh, s] layout
    (RoPE fused into the PSUM eviction; the hi/lo projection descale rides
    the host cos/sin tables).  Scores are computed transposed so exp tiles
    feed A@V directly; row-sums via a ones-vector matmul; reciprocal
    broadcast via a DRAM partition-broadcast roundtrip.
  - Causal-aware narrowing: per [128 sk, 512 sq] block the host finds the
    leading fully-masked column range; scores/AV/rowsum/exp all run on the
    [j0, 512) window only (j0 capped at 256 to keep fp32r matmuls >=256
    wide).  Mask blocks are classified allow/skip/mixed as in the baseline,
    so the kernel stays correct for arbitrary masks.
  - y is stored fp16 (the partial sums are small); host accumulates in fp32.
"""
import json
import math

import ml_dtypes
import numpy as np

import concourse.bass as bass
import concourse.mybir as mybir
import concourse.tile as tile
from concourse.bass_utils import run_bass_kernel_spmd

# ---------------------------------------------------------------- constants
B = 2
S = 2048
DIM = 2048
H = 16
KVH = 4
DH = 128
HPC = 4           # query heads per core
NCORE = 8
THETA = 10000.0
P = 128
ST = 512          # s-tile width (sequence) for projections / attention rhs
NCH = DIM // P    # 16 contraction chunks
NPAIR = NCH // 2  # 8 DoubleRow chunk pairs
NST = S // ST     # 4 s-tiles
SCALE = 1.0 / math.sqrt(DH)
F32 = mybir.dt.float32
F32R = mybir.dt.float32r
F16 = mybir.dt.float16
BF16 = mybir.dt.bfloat16
F8 = mybir.dt.float8e4
E4NP = ml_dtypes.float8_e4m3
DR = mybir.MatmulPerfMode.DoubleRow
NEG_THRESH = -1e30
OC = 4            # 2^OC scale on device-encoded attention-out codes
MAX_RESIDENT_MIXED = 24

# ------------------------------------------------- walrus multi-wait fixup
# This toolchain's walrus supports fewer sync-waits per instruction than
# Tile emits (observed: Matmult chokes at 2, Drain at 3).  Splitting excess
# waits onto NoOps on the same engine queue immediately before the
# instruction is semantically identical (the engine stalls at the NoOp).
_SKIP_OPCODES = {"CollectiveCompute"}


def _split_waits_in_bir(d: dict) -> None:
    for fn in d.get("functions", []):
        for blk in fn.get("blocks", []):
            out = []
            for inst in blk.get("instructions", []):
                si = inst.get("sync_info")
                waits = (si or {}).get("on_wait") or []
                if len(waits) > 1 and inst.get("opcode", "") not in _SKIP_OPCODES:
                    for k, w in enumerate(waits[1:]):
                        out.append({
                            "debug": inst.get("debug", 0),
                            "engine": inst["engine"],
                            "ins": [],
                            "name": f"{inst['name']}-wsplit{k}",
                            "opcode": "NoOp",
                            "outs": [],
                            "sync_info": {"on_update": [], "on_wait": [w]},
                        })
                    si["on_wait"] = waits[:1]
                out.append(inst)
            blk["instructions"] = out


_waitfix_installed = False


def _install_waitfix():
    global _waitfix_installed
    if _waitfix_installed:
        return
    orig = bass.Bass.to_json_bytes

    def to_json_bytes_split(self):
        d = json.loads(orig(self))
        _split_waits_in_bir(d)
        return json.dumps(d).encode()

    bass.Bass.to_json_bytes = to_json_bytes_split
    _waitfix_installed = True


# ------------------------------------------------------------ program build
def build_program(block_kind, scales, mask_exact_binary=True):
    """block_kind[(c, t)]: ('allow', j0) | 'skip' | ('mixed', idx, j0, j0a, j1a)
    for sk-chunk c x sq-tile t of the transposed mask; j0 = PE compute-window
    start (capped 256), [j0a, j1a) = mask-add window.
    scales: dict with log2 exponents ax/bqk/bv/bo (host code scales)."""
    _install_waitfix()
    from contextlib import ExitStack
    n_mixed = sum(1 for v in block_kind.values()
                  if isinstance(v, tuple) and v[0] == "mixed")
    resident = n_mixed <= MAX_RESIDENT_MIXED
    mdt = BF16 if mask_exact_binary else F32
    chunks_of = {}
    for t in range(NST):
        ch = [(c,) + ((0,) if block_kind[(c, t)][0] == "allow"
                      else block_kind[(c, t)][2:])
              for c in range(NCH) if block_kind[(c, t)] != "skip"]
        # first processed chunk must cover the full window so the PSUM
        # accumulation group starts every column
        if ch:
            ch[0] = (ch[0][0], 0) + ch[0][2:]
        chunks_of[t] = ch

    v_desc = 2.0 ** (-(scales["ax"] + scales["bv"]))
    y_desc = 2.0 ** (-(OC + scales["bo"]))

    nc = bass.Bass("TRN2", target_bir_lowering=False, debug=False)
    xh = nc.declare_dram_parameter("xh", [DIM, S], F8, isOutput=False)
    xl = nc.declare_dram_parameter("xl", [DIM, S], F8, isOutput=False)
    wqh = nc.declare_dram_parameter("wqh", [DIM, HPC * DH], F8, isOutput=False)
    wql = nc.declare_dram_parameter("wql", [DIM, HPC * DH], F8, isOutput=False)
    wkh = nc.declare_dram_parameter("wkh", [DIM, DH], F8, isOutput=False)
    wkl = nc.declare_dram_parameter("wkl", [DIM, DH], F8, isOutput=False)
    wvh = nc.declare_dram_parameter("wvh", [DIM, DH], F8, isOutput=False)
    wvl = nc.declare_dram_parameter("wvl", [DIM, DH], F8, isOutput=False)
    woh = nc.declare_dram_parameter("woh", [HPC * DH, DIM], F8, isOutput=False)
    wol = nc.declare_dram_parameter("wol", [HPC * DH, DIM], F8, isOutput=False)
    cosT = nc.declare_dram_parameter("cosT", [DH, S], F32, isOutput=False)
    sinTs = nc.declare_dram_parameter("sinTs", [DH, S], F32, isOutput=False)
    onescol = nc.declare_dram_parameter("onescol", [P, 1], F32R, isOutput=False)
    ident = nc.declare_dram_parameter("ident", [P, P], F32R, isOutput=False)
    if n_mixed:
        mmask = nc.declare_dram_parameter(
            "mmask", [n_mixed, P, ST], mdt, isOutput=False)
    y = nc.declare_dram_parameter("y", [S, DIM], F16, isOutput=True)
    rs_scr = nc.dram_tensor("rs_scr", [HPC * NST, 1, ST], F32)

    allp = mybir.AluOpType
    AF = mybir.ActivationFunctionType

    # pair-interleaved views: chunk pair j rows = j*256 + two*128 + p
    def pair_view(t3):
        return t3[:].rearrange("(j two p) m -> p j two m", two=2, p=P)

    with tile.TileContext(nc) as tc, ExitStack() as ctx:
        # ---- persistent pool -----------------------------------------
        keep = ctx.enter_context(tc.tile_pool(name="keep", bufs=1))
        ones_c = keep.tile([P, 1], F32R)
        qT = keep.tile([P, HPC, S], F32R)    # Q^T per head, RoPEd
        kT = keep.tile([P, S], F32R)         # K^T, RoPEd
        vsb = keep.tile([P, NCH, DH], F32R)  # V in [s, dh] chunks
        oh = keep.tile([P, HPC, S], F8)      # attention out^T hi code
        ol = keep.tile([P, HPC, S], F8)      # attention out^T lo code
        woh_sb = keep.tile([P, HPC, DIM], F8)
        wol_sb = keep.tile([P, HPC, DIM], F8)
        if n_mixed and resident:
            mm_sb = keep.tile([P, n_mixed, ST], mdt)

        # preload out-proj weights + constants up front (off critical path)
        nc.sync.dma_start(
            out=woh_sb[:], in_=woh[:].rearrange("(h p) n -> p h n", p=P))
        nc.sync.dma_start(
            out=wol_sb[:], in_=wol[:].rearrange("(h p) n -> p h n", p=P))
        nc.sync.dma_start(out=ones_c[:], in_=onescol[:])
        if n_mixed and resident:
            nc.sync.dma_start(
                out=mm_sb[:], in_=mmask[:].rearrange("n p m -> p n m"))

        # ---- projection phase ----------------------------------------
        with ExitStack() as pctx:
            wpool = pctx.enter_context(tc.tile_pool(name="wpool", bufs=1))
            wq_h = wpool.tile([P, NPAIR, 2, HPC * DH], F8)
            wq_l = wpool.tile([P, NPAIR, 2, HPC * DH], F8)
            wk_h = wpool.tile([P, NPAIR, 2, DH], F8)
            wk_l = wpool.tile([P, NPAIR, 2, DH], F8)
            wv_h = wpool.tile([P, NPAIR, 2, DH], F8)
            wv_l = wpool.tile([P, NPAIR, 2, DH], F8)
            for dst, src in ((wq_h, wqh), (wq_l, wql), (wk_h, wkh),
                             (wk_l, wkl), (wv_h, wvh), (wv_l, wvl)):
                nc.sync.dma_start(out=dst[:], in_=pair_view(src))
            id_sb = wpool.tile([P, P], F32R)
            nc.sync.dma_start(out=id_sb[:], in_=ident[:])
            vT = wpool.tile([P, S], F32R)    # V^T staging

            xpool = pctx.enter_context(tc.tile_pool(name="xpool", bufs=6))
            rpool = pctx.enter_context(tc.tile_pool(name="rope", bufs=4))
            tabp = pctx.enter_context(tc.tile_pool(name="tabs", bufs=2))

            def rope_evict(dst, ps, cos_t, sin_t):
                # dst = ps * cos + rotate_half(ps) * sin  (sign + hi/lo code
                # descale baked into the host tables).  Rotation via two
                # partition-shifted ACT copies straight out of PSUM; cos-mult
                # reads PSUM aligned on DVE.
                srot = rpool.tile([P, ST], F32, tag="srot", name="srot")
                nc.scalar.activation(
                    out=srot[0:64, :], in_=ps[64:128, :], func=AF.Copy)
                nc.scalar.activation(
                    out=srot[64:128, :], in_=ps[0:64, :], func=AF.Copy)
                nc.vector.tensor_tensor(
                    out=srot[:], in0=srot[:], in1=sin_t[:], op=allp.mult)
                with nc.allow_low_precision(reason="f32r 32-bit storage"):
                    nc.vector.tensor_tensor(
                        out=dst, in0=ps[:], in1=cos_t[:], op=allp.mult)
                    nc.vector.tensor_tensor(
                        out=dst, in0=dst, in1=srot[:], op=allp.add)

            with tc.tile_pool(name="pp", bufs=8, space="PSUM") as pp:
                for st in range(NST):
                    cols = bass.ts(st, ST)
                    cos_t = tabp.tile([P, ST], F32, tag="cos", name="cos_t")
                    nc.sync.dma_start(out=cos_t[:], in_=cosT[:, cols])
                    sin_t = tabp.tile([P, ST], F32, tag="sin", name="sin_t")
                    nc.sync.dma_start(out=sin_t[:], in_=sinTs[:, cols])
                    ps_list = [pp.tile([P, ST], F32, tag="proj",
                                       name=f"proj{j}")
                               for j in range(6)]
                    xh3 = pair_view(xh)
                    xl3 = pair_view(xl)
                    for j in range(NPAIR):
                        xth = xpool.tile([P, 2, ST], F8, tag="xh", name="xth")
                        (nc.gpsimd if j % 2 == 0 else nc.scalar).dma_start(
                            out=xth[:], in_=xh3[:, j, :, cols])
                        xtl = xpool.tile([P, 2, ST], F8, tag="xl", name="xtl")
                        (nc.scalar if j % 2 == 0 else nc.gpsimd).dma_start(
                            out=xtl[:], in_=xl3[:, j, :, cols])
                        start = j == 0
                        stop = j == NPAIR - 1
                        wslices = (
                            [(ps_list[hh],
                              (wq_h[:, j], wq_l[:, j]),
                              (hh * DH, (hh + 1) * DH)) for hh in range(HPC)]
                            + [(ps_list[4], (wk_h[:, j], wk_l[:, j]), (0, DH)),
                               (ps_list[5], (wv_h[:, j], wv_l[:, j]), (0, DH))]
                        )
                        for ps, (w_h, w_l), (m0, m1) in wslices:
                            nc.tensor.matmul(
                                ps[:], w_h[:, :, m0:m1], xth[:],
                                start=start, stop=False, perf_mode=DR)
                            nc.tensor.matmul(
                                ps[:], w_l[:, :, m0:m1], xth[:],
                                start=False, stop=False, perf_mode=DR)
                            nc.tensor.matmul(
                                ps[:], w_h[:, :, m0:m1], xtl[:],
                                start=False, stop=stop, perf_mode=DR)
                    for hh in range(HPC):
                        rope_evict(qT[:, hh, cols], ps_list[hh][:],
                                   cos_t, sin_t)
                    rope_evict(kT[:, cols], ps_list[4][:], cos_t, sin_t)
                    nc.scalar.activation(
                        out=vT[:, cols], in_=ps_list[5][:], func=AF.Copy,
                        scale=v_desc)

            # V^T -> V chunks via PE transpose
            with tc.tile_pool(name="pt", bufs=4, space="PSUM") as pt:
                for c in range(NCH):
                    tp = pt.tile([P, P], F32R, tag="tp", name="tp")
                    nc.tensor.transpose(
                        tp[:], vT[:, c * P:(c + 1) * P], id_sb[:])
                    nc.scalar.activation(
                        out=vsb[:, c, :], in_=tp[:], func=AF.Copy)

        # ---- attention phase -----------------------------------------
        with ExitStack() as actx:
            a2 = actx.enter_context(ExitStack())
            epool = a2.enter_context(tc.tile_pool(name="epool", bufs=8))
            tpool = a2.enter_context(tc.tile_pool(name="tpool", bufs=6))
            rsp = a2.enter_context(tc.tile_pool(name="rsp", bufs=4))
            if n_mixed and not resident:
                mstr = a2.enter_context(tc.tile_pool(name="mstr", bufs=4))
            pc = a2.enter_context(tc.tile_pool(name="pc", bufs=4, space="PSUM"))
            po = a2.enter_context(tc.tile_pool(name="po", bufs=2, space="PSUM"))
            pr = a2.enter_context(tc.tile_pool(name="pr", bufs=2, space="PSUM"))

            tiles = []
            for hh in range(HPC):
                for t in range(NST):
                    tiles.append({"hh": hh, "t": t,
                                  "chunks": chunks_of[t]})
            flat = [(ti, ci) for ti, td in enumerate(tiles)
                    for ci in range(len(td["chunks"]))]
            etiles = {}
            LOOKAHEAD = 3

            def produce(ti, ci):
                td = tiles[ti]
                info = td["chunks"][ci]
                c, j0 = info[0], info[1]
                t = td["t"]
                cs = slice(t * ST + j0, (t + 1) * ST)
                ps_c = pc.tile([P, ST], F32, tag="c", name="ps_c")
                nc.tensor.matmul(
                    ps_c[:, j0:], kT[:, c * P:(c + 1) * P],
                    qT[:, td["hh"], cs], start=True, stop=True)
                if len(info) == 4:
                    _, _, j0a, j1a = info
                    idx = block_kind[(c, t)][1]
                    if resident:
                        msl = mm_sb[:, idx, j0a:j1a]
                    else:
                        mtile = mstr.tile([P, ST], mdt, tag="ms",
                                          name="mtile")
                        nc.sync.dma_start(out=mtile[:], in_=mmask[idx])
                        msl = mtile[:, j0a:j1a]
                    nc.vector.tensor_tensor(
                        out=ps_c[:, j0a:j1a], in0=ps_c[:, j0a:j1a],
                        in1=msl, op=allp.add)
                et = epool.tile([P, ST], F32R, tag="e", name="et")
                nc.scalar.activation(
                    out=et[:, j0:], in_=ps_c[:, j0:], func=AF.Exp,
                    scale=SCALE)
                etiles[(ti, ci)] = et

            def consume(ti, ci):
                td = tiles[ti]
                info = td["chunks"][ci]
                c, j0 = info[0], info[1]
                if ci == 0:
                    td["ps_o"] = po.tile([P, ST], F32, tag="o", name="ps_o")
                    td["ps_r"] = pr.tile([1, ST], F32, tag="r", name="ps_r")
                et = etiles.pop((ti, ci))
                first = (ci == 0)
                last = (ci == len(td["chunks"]) - 1)
                nc.tensor.matmul(
                    td["ps_o"][:, j0:], vsb[:, c, :], et[:, j0:],
                    start=first, stop=last)
                nc.tensor.matmul(
                    td["ps_r"][:, j0:], ones_c[:], et[:, j0:],
                    start=first, stop=last)
                if last:
                    rs = rsp.tile([1, ST], F32, tag="rs", name="rs")
                    nc.vector.reciprocal(out=rs[:], in_=td["ps_r"][:])
                    slot = ti
                    nc.sync.dma_start(out=rs_scr[slot], in_=rs[:])
                    bc = tpool.tile([P, ST], F32, tag="bc", name="bc")
                    nc.sync.dma_start(
                        out=bc[:], in_=rs_scr[slot].partition_broadcast(P))
                    cols = bass.ts(td["t"], ST)
                    tmp = tpool.tile([P, ST], F32, tag="tmp", name="tmp")
                    nc.vector.tensor_tensor(
                        out=tmp[:], in0=td["ps_o"][:], in1=bc[:],
                        op=allp.mult)
                    with nc.allow_low_precision(reason="fp8 codes"):
                        nc.scalar.activation(
                            out=oh[:, td["hh"], cols], in_=tmp[:],
                            func=AF.Copy)
                        nc.vector.tensor_tensor(
                            out=ol[:, td["hh"], cols], in0=tmp[:],
                            in1=oh[:, td["hh"], cols], op=allp.subtract)

            np_ = 0
            for i in range(len(flat)):
                while np_ < min(i + 1 + LOOKAHEAD, len(flat)):
                    produce(*flat[np_])
                    np_ += 1
                consume(*flat[i])
            # ---- output projection -----------------------------------
            a2.close()
            ypool = actx.enter_context(tc.tile_pool(name="ypool", bufs=4))
            with tc.tile_pool(name="py", bufs=4, space="PSUM") as py:
                for tq in range(S // P):
                    rows = bass.ts(tq, P)
                    for n in range(DIM // ST):
                        ncols = bass.ts(n, ST)
                        ps_y = py.tile([P, ST], F32, tag="y", name="ps_y")
                        for pidx in range(HPC // 2):
                            hsl = slice(2 * pidx, 2 * pidx + 2)
                            start = pidx == 0
                            stop = pidx == HPC // 2 - 1
                            nc.tensor.matmul(
                                ps_y[:], oh[:, hsl, rows], woh_sb[:, hsl, ncols],
                                start=start, stop=False, perf_mode=DR)
                            nc.tensor.matmul(
                                ps_y[:], ol[:, hsl, rows], woh_sb[:, hsl, ncols],
                                start=False, stop=False, perf_mode=DR)
                            nc.tensor.matmul(
                                ps_y[:], oh[:, hsl, rows], wol_sb[:, hsl, ncols],
                                start=False, stop=stop, perf_mode=DR)
                        ysb = ypool.tile([P, ST], F16, tag="ys", name="ysb")
                        with nc.allow_low_precision(reason="f16 partials"):
                            if (tq * (DIM // ST) + n) % 2 == 0:
                                nc.vector.tensor_scalar_mul(
                                    ysb[:], ps_y[:], y_desc)
                            else:
                                nc.scalar.activation(
                                    out=ysb[:], in_=ps_y[:], func=AF.Copy,
                                    scale=y_desc)
                        yq = nc.gpsimd if n % 2 == 0 else nc.scalar
                        yq.dma_start(
                            out=y[tq * P:(tq + 1) * P, n * ST:(n + 1) * ST],
                            in_=ysb[:])
    return nc


# ------------------------------------------------------------- host driver
def _classify_mask(mask2d):
    """Classify [128, 512] blocks of mask^T.

    Returns (block_kind, mixed_vals, exact_binary): block_kind[(c, t)] is
    ('allow', 0) | 'skip' | ('mixed', idx, j0, j0a, j1a) where j0 is the PE
    compute-window start (leading fully-masked columns, capped at 256) and
    [j0a, j1a) the mask-add window; mixed_vals is [n_mixed, 128, 512] of
    mask^T values pre-scaled by sqrt(DH), clipped to stay finite;
    exact_binary is True when every mixed value is 0 or <= -1e30."""
    mT = mask2d.T  # [sk, sq]
    block_kind = {}
    mixed = []
    exact_binary = True
    for c in range(NCH):
        for t in range(NST):
            blk = mT[c * P:(c + 1) * P, t * ST:(t + 1) * ST]
            if not blk.any():
                block_kind[(c, t)] = ("allow", 0)
            elif (blk <= NEG_THRESH).all():
                block_kind[(c, t)] = "skip"
            else:
                allmasked = (blk <= NEG_THRESH).all(axis=0)
                j0 = 0
                while j0 < ST and allmasked[j0]:
                    j0 += 1
                j0 = min(j0, ST - 256, 256)
                j0 = max(j0, 0)
                nz = np.flatnonzero(blk.any(axis=0))
                j0a, j1a = int(nz[0]), int(nz[-1]) + 1
                j0a = max(j0a, j0)
                if j1a <= j0a:
                    block_kind[(c, t)] = ("allow", j0)
                    continue
                block_kind[(c, t)] = ("mixed", len(mixed), j0, j0a, j1a)
                if not ((blk == 0) | (blk <= NEG_THRESH)).all():
                    exact_binary = False
                scaled = np.clip(
                    blk.astype(np.float64) * math.sqrt(DH), -3e38, 3e38
                ).astype(np.float32)
                mixed.append(scaled)
    mv = np.stack(mixed) if mixed else None
    return block_kind, mv, exact_binary


def _rope_tables(position_ids, desc):
    pos = position_ids.reshape(-1).astype(np.float64)  # [S]
    inv = 1.0 / (THETA ** (np.arange(0, DH, 2, dtype=np.float64) / DH))
    fr = pos[None, :] * inv[:, None]          # [64, S]
    cosT = np.empty((DH, S), np.float32)
    sinTs = np.empty((DH, S), np.float32)
    cosT[0:64] = np.cos(fr) * desc
    cosT[64:128] = np.cos(fr) * desc
    sinTs[0:64] = -np.sin(fr) * desc
    sinTs[64:128] = np.sin(fr) * desc
    return cosT, sinTs


def _enc3(t, target=120.0):
    """two-term e4m3 expansion with power-of-2 prescale; returns
    (hi, lo, exp2) with hi + lo ~= t * 2^exp2."""
    t = np.asarray(t, np.float32)
    amax = float(np.abs(t).max())
    e = int(np.floor(np.log2(target / max(amax, 1e-30))))
    ts = t * (2.0 ** e)
    hi = ts.astype(E4NP)
    lo = (ts - hi.astype(np.float32)).astype(E4NP)
    return hi, lo, e


def _prep_inputs(x, mask, position_ids, Wq, Wk, Wv, Wo):
    x = np.asarray(x, np.float32)
    Wq = np.asarray(Wq, np.float32)
    Wk = np.asarray(Wk, np.float32)
    Wv = np.asarray(Wv, np.float32)
    Wo = np.asarray(Wo, np.float32)
    block_kind, mixed_vals, exact_binary = _classify_mask(
        np.asarray(mask)[0, 0])
    if mixed_vals is not None and exact_binary:
        mixed_vals = mixed_vals.astype(ml_dtypes.bfloat16)

    # global power-2 scales shared by all cores (compiled constants)
    ax = int(np.floor(np.log2(120.0 / max(float(np.abs(x).max()), 1e-30))))
    sqk = 2.0 ** int(np.floor(np.log2(
        120.0 / max(float(np.abs(Wq).max()), float(np.abs(Wk).max()), 1e-30))))
    bqk = int(np.log2(sqk))
    bv = int(np.floor(np.log2(120.0 / max(float(np.abs(Wv).max()), 1e-30))))
    bo = int(np.floor(np.log2(120.0 / max(float(np.abs(Wo).max()), 1e-30))))
    scales = {"ax": ax, "bqk": bqk, "bv": bv, "bo": bo}

    def enc_fixed(t, e):
        ts = np.asarray(t, np.float32) * (2.0 ** e)
        hi = ts.astype(E4NP)
        lo = (ts - hi.astype(np.float32)).astype(E4NP)
        return np.ascontiguousarray(hi), np.ascontiguousarray(lo)

    desc = 2.0 ** (-(ax + bqk))
    cosT, sinTs = _rope_tables(np.asarray(position_ids), desc)
    onescol = np.full((P, 1), 2.0 ** (-OC), np.float32)
    ident = np.eye(P, dtype=np.float32)

    in_maps = []
    for core in range(NCORE):
        b, g = divmod(core, KVH)
        xbh, xbl = enc_fixed(x[b].T, ax)
        qh_, ql_ = enc_fixed(Wq[:, g * HPC * DH:(g + 1) * HPC * DH], bqk)
        kh_, kl_ = enc_fixed(Wk[:, g * DH:(g + 1) * DH], bqk)
        vh_, vl_ = enc_fixed(Wv[:, g * DH:(g + 1) * DH], bv)
        oh_, ol_ = enc_fixed(Wo[g * HPC * DH:(g + 1) * HPC * DH, :], bo)
        m = {
            "xh": xbh, "xl": xbl,
            "wqh": qh_, "wql": ql_,
            "wkh": kh_, "wkl": kl_,
            "wvh": vh_, "wvl": vl_,
            "woh": oh_, "wol": ol_,
            "cosT": cosT, "sinTs": sinTs,
            "onescol": onescol, "ident": ident,
        }
        if mixed_vals is not None:
            m["mmask"] = mixed_vals
        in_maps.append(m)
    return (block_kind, scales, exact_binary), in_maps


def kernel(x, mask, position_ids, Wq, Wk, Wv, Wo):
    (block_kind, scales, exact_binary), in_maps = _prep_inputs(
        x, mask, position_ids, Wq, Wk, Wv, Wo)
    nc = build_program(block_kind, scales, mask_exact_binary=exact_binary)
    res = run_bass_kernel_spmd(nc, in_maps, core_ids=list(range(NCORE)))
    out = np.zeros((B, S, DIM), np.float32)
    for core in range(NCORE):
        b = core // KVH
        out[b] += res.results[core]["y"].astype(np.float32)
    return out


# revision 26
# speedup vs baseline: 36215.4011x; 36215.4011x over previous
"""GQA attention block (RoPE + causal attention + out-proj) on 8 TRN2 cores.

Problem: nn_AdvancedAttn (B=2, S=2048, DIM=2048, H=16 q-heads, KVH=4 kv-heads,
DH=128), fp32 in/out.

Sharding: core (b, g) for b in {0,1}, g in {0..3} handles batch b and kv-head
group g (4 query heads + 1 kv head).  Wq/Wk/Wv are split along the head dim,
Wo along its input dim; the 4 partial Wo outputs per batch are summed on host
(the all-reduce of tensor parallelism).

Device kernel (per core):
  - The Q/K/V and output projections run as fp8 DoubleRow matmuls (0.5
    cycles/row) on a 3-product hi/lo expansion: each operand is split into
    two e4m3 codes (hi = e4m3(t), lo = e4m3(t - hi)) and the product
    (hi+lo)x(hi+lo) is computed dropping the lo*lo term.  This costs 0.75
    cycles/row instead of fp32r's 1.0 with ~tf32 accuracy (the dropped term
    is ~2^-10 relative).  Chunk pairs ride the two DoubleRow k-tiles, so no
    operand duplication is needed.  x / W codes are prepared on host; the
    attention-output codes are produced on device during softmax
    normalization.
  - Scores / AV / rowsum stay fp32r with Q^T/K^T produced in [dh, s] layout
    (RoPE fused into the PSUM eviction; the hi/lo projection descale rides
    the host cos/sin tables).  Scores are computed transposed so exp tiles
    feed A@V directly; row-sums via a ones-vector matmul; reciprocal
    broadcast via a DRAM partition-broadcast roundtrip.
  - Causal-aware narrowing: per [128 sk, 512 sq] block the host finds the
    leading fully-masked column range; scores/AV/rowsum/exp all run on the
    [j0, 512) window only (j0 capped at 256 to keep fp32r matmuls >=256
    wide).  Mask blocks are classified allow/skip/mixed as in the baseline,
    so the kernel stays correct for arbitrary masks.
  - y is stored fp16 (the partial sums are small); host accumulates in fp32.
"""
import json
import math

import ml_dtypes
import numpy as np

import concourse.bass as bass
import concourse.mybir as mybir
import concourse.tile as tile
from concourse.bass_utils import run_bass_kernel_spmd

# ---------------------------------------------------------------- constants
B = 2
S = 2048
DIM = 2048
H = 16
KVH = 4
DH = 128
HPC = 4           # query heads per core
NCORE = 8
THETA = 10000.0
P = 128
ST = 512          # s-tile width (sequence) for projections / attention rhs
NCH = DIM // P    # 16 contraction chunks
NPAIR = NCH // 2  # 8 DoubleRow chunk pairs
NST = S // ST     # 4 s-tiles
SCALE = 1.0 / math.sqrt(DH)
F32 = mybir.dt.float32
F32R = mybir.dt.float32r
F16 = mybir.dt.float16
BF16 = mybir.dt.bfloat16
F8 = mybir.dt.float8e4
E4NP = ml_dtypes.float8_e4m3
DR = mybir.MatmulPerfMode.DoubleRow
NEG_THRESH = -1e30
OC = 4            # 2^OC scale on device-encoded attention-out codes
MAX_RESIDENT_MIXED = 24

# ------------------------------------------------- walrus multi-wait fixup
# This toolchain's walrus supports fewer sync-waits per instruction than
# Tile emits (observed: Matmult chokes at 2, Drain at 3).  Splitting excess
# waits onto NoOps on the same engine queue immediately before the
# instruction is semantically identical (the engine stalls at the NoOp).
_SKIP_OPCODES = {"CollectiveCompute"}


def _split_waits_in_bir(d: dict) -> None:
    for fn in d.get("functions", []):
        for blk in fn.get("blocks", []):
            out = []
            for inst in blk.get("instructions", []):
                si = inst.get("sync_info")
                waits = (si or {}).get("on_wait") or []
                if len(waits) > 1 and inst.get("opcode", "") not in _SKIP_OPCODES:
                    for k, w in enumerate(waits[1:]):
                        out.append({
                            "debug": inst.get("debug", 0),
                            "engine": inst["engine"],
                            "ins": [],
                            "name": f"{inst['name']}-wsplit{k}",
                            "opcode": "NoOp",
                            "outs": [],
                            "sync_info": {"on_update": [], "on_wait": [w]},
                        })
                    si["on_wait"] = waits[:1]
                out.append(inst)
            blk["instructions"] = out


_waitfix_installed = False


def _install_waitfix():
    global _waitfix_installed
    if _waitfix_installed:
        return
    orig = bass.Bass.to_json_bytes

    def to_json_bytes_split(self):
        d = json.loads(orig(self))
        _split_waits_in_bir(d)
        return json.dumps(d).encode()

    bass.Bass.to_json_bytes = to_json_bytes_split
    _waitfix_installed = True


# ------------------------------------------------------------ program build
def build_program(block_kind, scales, mask_exact_binary=True):
    """block_kind[(c, t)]: ('allow', j0) | 'skip' | ('mixed', idx, j0, j0a, j1a)
    for sk-chunk c x sq-tile t of the transposed mask; j0 = PE compute-window
    start (capped 256), [j0a, j1a) = mask-add window.
    scales: dict with log2 exponents ax/bqk/bv/bo (host code scales)."""
    _install_waitfix()
    from contextlib import ExitStack
    n_mixed = sum(1 for v in block_kind.values()
                  if isinstance(v, tuple) and v[0] == "mixed")
    resident = n_mixed <= MAX_RESIDENT_MIXED
    mdt = BF16 if mask_exact_binary else F32
    chunks_of = {}
    for t in range(NST):
        ch = [(c,) + ((0,) if block_kind[(c, t)][0] == "allow"
                      else block_kind[(c, t)][2:])
              for c in range(NCH) if block_kind[(c, t)] != "skip"]
        # first processed chunk must cover the full window so the PSUM
        # accumulation group starts every column
        if ch:
            ch[0] = (ch[0][0], 0) + ch[0][2:]
        chunks_of[t] = ch

    v_desc = 2.0 ** (-(scales["ax"] + scales["bv"]))
    y_desc = 2.0 ** (-(OC + scales["bo"]))

    nc = bass.Bass("TRN2", target_bir_lowering=False, debug=False)
    xh = nc.declare_dram_parameter("xh", [DIM, S], F8, isOutput=False)
    xl = nc.declare_dram_parameter("xl", [DIM, S], F8, isOutput=False)
    wqh = nc.declare_dram_parameter("wqh", [DIM, HPC * DH], F8, isOutput=False)
    wql = nc.declare_dram_parameter("wql", [DIM, HPC * DH], F8, isOutput=False)
    wkh = nc.declare_dram_parameter("wkh", [DIM, DH], F8, isOutput=False)
    wkl = nc.declare_dram_parameter("wkl", [DIM, DH], F8, isOutput=False)
    wvh = nc.declare_dram_parameter("wvh", [DIM, DH], F8, isOutput=False)
    wvl = nc.declare_dram_parameter("wvl", [DIM, DH], F8, isOutput=False)
    woh = nc.declare_dram_parameter("woh", [HPC * DH, DIM], F8, isOutput=False)
    wol = nc.declare_dram_parameter("wol", [HPC * DH, DIM], F8, isOutput=False)
    cstab = nc.declare_dram_parameter(
        "cstab", [DH, NST, 2, ST], F32, isOutput=False)
    onescol = nc.declare_dram_parameter("onescol", [P, 1], F32R, isOutput=False)
    ident = nc.declare_dram_parameter("ident", [P, P], F32R, isOutput=False)
    if n_mixed:
        mmask = nc.declare_dram_parameter(
            "mmask", [n_mixed, P, ST], mdt, isOutput=False)
    y = nc.declare_dram_parameter("y", [S, DIM], F16, isOutput=True)
    rs_scr = nc.dram_tensor("rs_scr", [HPC * NST, 1, ST], F32)

    allp = mybir.AluOpType
    AF = mybir.ActivationFunctionType

    # pair-interleaved views: chunk pair j rows = j*256 + two*128 + p
    def pair_view(t3):
        return t3[:].rearrange("(j two p) m -> p j two m", two=2, p=P)

    with tile.TileContext(nc) as tc, ExitStack() as ctx:
        # ---- persistent pool -----------------------------------------
        keep = ctx.enter_context(tc.tile_pool(name="keep", bufs=1))
        ones_c = keep.tile([P, 1], F32R)
        qT = keep.tile([P, HPC, S], F32R)    # Q^T per head, RoPEd
        kT = keep.tile([P, S], F32R)         # K^T, RoPEd
        vsb = keep.tile([P, NCH, DH], F32R)  # V in [s, dh] chunks
        oh = keep.tile([P, HPC, S], F8)      # attention out^T hi code
        ol = keep.tile([P, HPC, S], F8)      # attention out^T lo code
        woh_sb = keep.tile([P, HPC, DIM], F8)
        wol_sb = keep.tile([P, HPC, DIM], F8)
        if n_mixed and resident:
            mm_sb = keep.tile([P, n_mixed, ST], mdt)

        # ---- projection phase ----------------------------------------
        with ExitStack() as pctx:
            wpool = pctx.enter_context(tc.tile_pool(name="wpool", bufs=1))
            wq_h = wpool.tile([P, NPAIR, 2, HPC * DH], F8)
            wq_l = wpool.tile([P, NPAIR, 2, HPC * DH], F8)
            wk_h = wpool.tile([P, NPAIR, 2, DH], F8)
            wk_l = wpool.tile([P, NPAIR, 2, DH], F8)
            wv_h = wpool.tile([P, NPAIR, 2, DH], F8)
            wv_l = wpool.tile([P, NPAIR, 2, DH], F8)
            # v/k weights first (first outputs computed), wq in halves
            for dst, src in ((wv_h, wvh), (wv_l, wvl), (wk_h, wkh),
                             (wk_l, wkl)):
                nc.sync.dma_start(out=dst[:], in_=pair_view(src))
            for half in range(2):
                hs = slice(half * (NPAIR // 2), (half + 1) * (NPAIR // 2))
                nc.sync.dma_start(out=wq_h[:, hs], in_=pair_view(wqh)[:, hs])
                nc.sync.dma_start(out=wq_l[:, hs], in_=pair_view(wql)[:, hs])
            id_sb = wpool.tile([P, P], F32R)
            nc.sync.dma_start(out=id_sb[:], in_=ident[:])
            vT = wpool.tile([P, S], F32R)    # V^T staging

            # out-proj weights + mask blocks: needed much later; sharded
            # transfers on the vector queue, spread across st iterations so
            # they never front-run the projection-critical DMAs
            woh4 = woh[:].rearrange("(h p) n -> p h n", p=P)
            wol4 = wol[:].rearrange("(h p) n -> p h n", p=P)
            deferred = [(ones_c[:], onescol[:])]
            for hh in range(HPC):
                deferred.append((woh_sb[:, hh], woh4[:, hh]))
                deferred.append((wol_sb[:, hh], wol4[:, hh]))
            if n_mixed and resident:
                deferred.append(
                    (mm_sb[:], mmask[:].rearrange("n p m -> p n m")))

            xpool = pctx.enter_context(tc.tile_pool(name="xpool", bufs=2))
            rpool = pctx.enter_context(tc.tile_pool(name="rope", bufs=4))
            tabp = pctx.enter_context(tc.tile_pool(name="tabs", bufs=2))

            def rope_evict(dst, ps, cos_t, sin_t):
                # dst = ps * cos + rotate_half(ps) * sin  (sign + hi/lo code
                # descale baked into the host tables).  Rotation via two
                # partition-shifted ACT copies straight out of PSUM; cos-mult
                # reads PSUM aligned on DVE.
                srot = rpool.tile([P, ST], F32, tag="srot", name="srot")
                nc.scalar.activation(
                    out=srot[0:64, :], in_=ps[64:128, :], func=AF.Copy)
                nc.scalar.activation(
                    out=srot[64:128, :], in_=ps[0:64, :], func=AF.Copy)
                nc.vector.tensor_tensor(
                    out=srot[:], in0=srot[:], in1=sin_t[:], op=allp.mult)
                with nc.allow_low_precision(reason="f32r 32-bit storage"):
                    nc.vector.tensor_tensor(
                        out=dst, in0=ps[:], in1=cos_t[:], op=allp.mult)
                    nc.vector.tensor_tensor(
                        out=dst, in0=dst, in1=srot[:], op=allp.add)

            pp = pctx.enter_context(
                tc.tile_pool(name="pp", bufs=6, space="PSUM"))
            pt = pctx.enter_context(
                tc.tile_pool(name="pt", bufs=2, space="PSUM"))
            xh3 = pair_view(xh)
            xl3 = pair_view(xl)
            xtiles = {}

            def load_x(st, split=1):
                cols = bass.ts(st, ST)
                xth = xpool.tile([P, NPAIR, 2, ST], F8, tag="xh", name="xth")
                xtl = xpool.tile([P, NPAIR, 2, ST], F8, tag="xl", name="xtl")
                q = NPAIR // split
                for i in range(split):
                    js = slice(i * q, (i + 1) * q)
                    nc.gpsimd.dma_start(
                        out=xth[:, js], in_=xh3[:, js, :, cols])
                    nc.scalar.dma_start(
                        out=xtl[:, js], in_=xl3[:, js, :, cols])
                xtiles[st] = (xth, xtl)

            load_x(0, split=4)
            for st in range(NST):
                cols = bass.ts(st, ST)
                cs_t = tabp.tile([P, 2, ST], F32, tag="cs", name="cs_t")
                nc.sync.dma_start(out=cs_t[:], in_=cstab[:, st])
                cos_t = cs_t[:, 0]
                sin_t = cs_t[:, 1]
                ps_list = [pp.tile([P, ST], F32, tag="proj", name=f"proj{j}")
                           for j in range(6)]
                xth, xtl = xtiles.pop(st)
                if st + 1 < NST:
                    load_x(st + 1)
                for _ in range(3):  # drain deferred preloads each s-tile
                    if deferred:
                        dst, src = deferred.pop(0)
                        nc.sync.dma_start(out=dst, in_=src)

                # output-major: v, k first so their PSUM banks recycle early
                # and the per-st transposes / rope chains overlap the q mms
                wslices = (
                    [(ps_list[5], (wv_h, wv_l), (0, DH)),
                     (ps_list[4], (wk_h, wk_l), (0, DH))]
                    + [(ps_list[hh], (wq_h, wq_l),
                        (hh * DH, (hh + 1) * DH)) for hh in range(HPC)]
                )
                for oi, (ps, (w_h, w_l), (m0, m1)) in enumerate(wslices):
                    for j in range(NPAIR):
                        nc.tensor.matmul(
                            ps[:], w_h[:, j, :, m0:m1], xth[:, j],
                            start=j == 0, stop=False, perf_mode=DR)
                        nc.tensor.matmul(
                            ps[:], w_l[:, j, :, m0:m1], xth[:, j],
                            start=False, stop=False, perf_mode=DR)
                        nc.tensor.matmul(
                            ps[:], w_h[:, j, :, m0:m1], xtl[:, j],
                            start=False, stop=j == NPAIR - 1, perf_mode=DR)
                    if oi == 0:
                        nc.scalar.activation(
                            out=vT[:, cols], in_=ps[:], func=AF.Copy,
                            scale=v_desc)
                    elif oi == 1:
                        rope_evict(kT[:, cols], ps[:], cos_t, sin_t)
                        # V^T -> V chunks via PE transpose, per s-tile;
                        # emitted after k's mms so PE never waits the evict
                        for cc in range(ST // P):
                            c = st * (ST // P) + cc
                            tp = pt.tile([P, P], F32R, tag="tp", name="tp")
                            nc.tensor.transpose(
                                tp[:], vT[:, c * P:(c + 1) * P], id_sb[:])
                            nc.vector.tensor_copy(vsb[:, c, :], tp[:])
                    else:
                        rope_evict(qT[:, oi - 2, cols], ps[:], cos_t, sin_t)
            while deferred:
                dst, src = deferred.pop(0)
                nc.sync.dma_start(out=dst, in_=src)

        # ---- attention + out-projection, interleaved per s-tile ------
        with ExitStack() as actx:
            epool = actx.enter_context(tc.tile_pool(name="epool", bufs=8))
            tpool = actx.enter_context(tc.tile_pool(name="tpool", bufs=4))
            rsp = actx.enter_context(tc.tile_pool(name="rsp", bufs=4))
            ypool = actx.enter_context(tc.tile_pool(name="ypool", bufs=4))
            if n_mixed and not resident:
                mstr = actx.enter_context(tc.tile_pool(name="mstr", bufs=4))
            pc = actx.enter_context(
                tc.tile_pool(name="pc", bufs=2, space="PSUM"))
            po = actx.enter_context(
                tc.tile_pool(name="po", bufs=2, space="PSUM"))
            pr = actx.enter_context(
                tc.tile_pool(name="pr", bufs=2, space="PSUM"))
            py = actx.enter_context(
                tc.tile_pool(name="py", bufs=2, space="PSUM"))

            tiles = []
            for t in range(NST):
                for hh in range(HPC):
                    tiles.append({"hh": hh, "t": t,
                                  "chunks": chunks_of[t]})
            flat = [(ti, ci) for ti, td in enumerate(tiles)
                    for ci in range(len(td["chunks"]))]
            etiles = {}
            LOOKAHEAD = 5

            def produce(ti, ci):
                td = tiles[ti]
                info = td["chunks"][ci]
                c, j0 = info[0], info[1]
                t = td["t"]
                cs = slice(t * ST + j0, (t + 1) * ST)
                ps_c = pc.tile([P, ST], F32, tag="c", name="ps_c")
                nc.tensor.matmul(
                    ps_c[:, j0:], kT[:, c * P:(c + 1) * P],
                    qT[:, td["hh"], cs], start=True, stop=True)
                if len(info) == 4:
                    _, _, j0a, j1a = info
                    idx = block_kind[(c, t)][1]
                    if resident:
                        msl = mm_sb[:, idx, j0a:j1a]
                    else:
                        mtile = mstr.tile([P, ST], mdt, tag="ms",
                                          name="mtile")
                        nc.sync.dma_start(out=mtile[:], in_=mmask[idx])
                        msl = mtile[:, j0a:j1a]
                    nc.vector.tensor_tensor(
                        out=ps_c[:, j0a:j1a], in0=ps_c[:, j0a:j1a],
                        in1=msl, op=allp.add)
                et = epool.tile([P, ST], F32R, tag="e", name="et")
                nc.scalar.activation(
                    out=et[:, j0:], in_=ps_c[:, j0:], func=AF.Exp,
                    scale=SCALE)
                etiles[(ti, ci)] = et

            def consume(ti, ci):
                td = tiles[ti]
                info = td["chunks"][ci]
                c, j0 = info[0], info[1]
                hh = td["hh"]
                if ci == 0:
                    td["ps_o"] = po.tile([P, ST], F32, tag="o", name="ps_o")
                    td["ps_r"] = pr.tile([1, ST], F32, tag="r", name="ps_r")
                ps_r = td["ps_r"][:]
                et = etiles.pop((ti, ci))
                first = (ci == 0)
                last = (ci == len(td["chunks"]) - 1)
                nc.tensor.matmul(
                    td["ps_o"][:, j0:], vsb[:, c, :], et[:, j0:],
                    start=first, stop=last)
                nc.tensor.matmul(
                    ps_r[:, j0:], ones_c[:], et[:, j0:],
                    start=first, stop=last)
                if last:
                    # reciprocal now (so the pr slot frees promptly); the
                    # broadcast + normalize are deferred one tile so the PE
                    # outer product never waits on the DVE reciprocal
                    rs = rsp.tile([1, ST], F32, tag="rs", name="rs")
                    nc.vector.reciprocal(out=rs[:], in_=ps_r)
                    td["rs"] = rs

            def emit_norm(ti):
                td = tiles[ti]
                nc.sync.dma_start(out=rs_scr[ti], in_=td["rs"][:])
                bc = tpool.tile([P, ST], F32, tag="bc", name="bc")
                nc.sync.dma_start(
                    out=bc[:], in_=rs_scr[ti].partition_broadcast(P))
                cols = bass.ts(td["t"], ST)
                hh = td["hh"]
                tmp = tpool.tile([P, ST], F32, tag="tmp", name="tmp")
                nc.vector.tensor_tensor(
                    out=tmp[:], in0=td["ps_o"][:], in1=bc[:], op=allp.mult)
                with nc.allow_low_precision(reason="fp8 codes"):
                    nc.vector.tensor_copy(oh[:, hh, cols], tmp[:])
                    nc.vector.tensor_tensor(
                        out=ol[:, hh, cols], in0=tmp[:],
                        in1=oh[:, hh, cols], op=allp.subtract)
            def outproj(t):
                # y rows of s-tile t: all 4 heads' oh/ol just finished
                for tq in range(t * (ST // P), (t + 1) * (ST // P)):
                    rows = bass.ts(tq, P)
                    ysb = None
                    for n in range(DIM // ST):
                        ncols = bass.ts(n, ST)
                        ps_y = py.tile([P, ST], F32, tag="y", name="ps_y")
                        for pidx in range(HPC // 2):
                            hsl = slice(2 * pidx, 2 * pidx + 2)
                            nc.tensor.matmul(
                                ps_y[:], oh[:, hsl, rows],
                                woh_sb[:, hsl, ncols],
                                start=pidx == 0, stop=False, perf_mode=DR)
                            nc.tensor.matmul(
                                ps_y[:], ol[:, hsl, rows],
                                woh_sb[:, hsl, ncols],
                                start=False, stop=False, perf_mode=DR)
                            nc.tensor.matmul(
                                ps_y[:], oh[:, hsl, rows],
                                wol_sb[:, hsl, ncols],
                                start=False, stop=pidx == HPC // 2 - 1,
                                perf_mode=DR)
                        if ysb is None:
                            ysb = ypool.tile([P, 2, ST], F16, tag="ys",
                                             name="ysb")
                        half = n % 2
                        with nc.allow_low_precision(reason="f16 partials"):
                            if n % 2 == 0:
                                nc.vector.tensor_scalar_mul(
                                    ysb[:, half], ps_y[:], y_desc)
                            else:
                                nc.scalar.activation(
                                    out=ysb[:, half], in_=ps_y[:],
                                    func=AF.Copy, scale=y_desc)
                        if half == 1:
                            yq = nc.gpsimd if n % 4 == 1 else nc.scalar
                            yq.dma_start(
                                out=y[tq * P:(tq + 1) * P,
                                      (n - 1) * ST:(n + 1) * ST],
                                in_=ysb[:])
                            ysb = None

            np_ = 0
            pending = []
            for i in range(len(flat)):
                while np_ < min(i + 1 + LOOKAHEAD, len(flat)):
                    produce(*flat[np_])
                    np_ += 1
                consume(*flat[i])
                ti, ci = flat[i]
                td = tiles[ti]
                if ci == len(td["chunks"]) - 1:
                    pending.append(ti)
                    if len(pending) > 1:
                        pti = pending.pop(0)
                        emit_norm(pti)
                        if tiles[pti]["hh"] == HPC - 1:
                            outproj(tiles[pti]["t"])
            for pti in pending:
                emit_norm(pti)
                if tiles[pti]["hh"] == HPC - 1:
                    outproj(tiles[pti]["t"])
    return nc


# ------------------------------------------------------------- host driver
def _classify_mask(mask2d):
    """Classify [128, 512] blocks of mask^T.

    Returns (block_kind, mixed_vals, exact_binary): block_kind[(c, t)] is
    ('allow', 0) | 'skip' | ('mixed', idx, j0, j0a, j1a) where j0 is the PE
    compute-window start (leading fully-masked columns, capped at 256) and
    [j0a, j1a) the mask-add window; mixed_vals is [n_mixed, 128, 512] of
    mask^T values pre-scaled by sqrt(DH), clipped to stay finite;
    exact_binary is True when every mixed value is 0 or <= -1e30."""
    mT = mask2d.T  # [sk, sq]
    block_kind = {}
    mixed = []
    exact_binary = True
    for c in range(NCH):
        for t in range(NST):
            blk = mT[c * P:(c + 1) * P, t * ST:(t + 1) * ST]
            if not blk.any():
                block_kind[(c, t)] = ("allow", 0)
            elif (blk <= NEG_THRESH).all():
                block_kind[(c, t)] = "skip"
            else:
                allmasked = (blk <= NEG_THRESH).all(axis=0)
                j0 = 0
                while j0 < ST and allmasked[j0]:
                    j0 += 1
                j0 = min(j0, ST - 256, 256)
                j0 = max(j0, 0)
                nz = np.flatnonzero(blk.any(axis=0))
                j0a, j1a = int(nz[0]), int(nz[-1]) + 1
                j0a = max(j0a, j0)
                if j1a <= j0a:
                    block_kind[(c, t)] = ("allow", j0)
                    continue
                block_kind[(c, t)] = ("mixed", len(mixed), j0, j0a, j1a)
                if not ((blk == 0) | (blk <= NEG_THRESH)).all():
                    exact_binary = False
                scaled = np.clip(
                    blk.astype(np.float64) * math.sqrt(DH), -3e38, 3e38
                ).astype(np.float32)
                mixed.append(scaled)
    mv = np.stack(mixed) if mixed else None
    return block_kind, mv, exact_binary


def _rope_tables(position_ids, desc):
    """packed [DH, NST, 2, ST]: [..., 0, :] = cos, [..., 1, :] = signed sin,
    both pre-scaled by the projection-code descale."""
    pos = position_ids.reshape(-1).astype(np.float64)  # [S]
    inv = 1.0 / (THETA ** (np.arange(0, DH, 2, dtype=np.float64) / DH))
    fr = pos[None, :] * inv[:, None]          # [64, S]
    cosT = np.empty((DH, S), np.float32)
    sinTs = np.empty((DH, S), np.float32)
    cosT[0:64] = np.cos(fr) * desc
    cosT[64:128] = np.cos(fr) * desc
    sinTs[0:64] = -np.sin(fr) * desc
    sinTs[64:128] = np.sin(fr) * desc
    cstab = np.stack([cosT.reshape(DH, NST, ST),
                      sinTs.reshape(DH, NST, ST)], axis=2)
    return np.ascontiguousarray(cstab)


def _enc3(t, target=120.0):
    """two-term e4m3 expansion with power-of-2 prescale; returns
    (hi, lo, exp2) with hi + lo ~= t * 2^exp2."""
    t = np.asarray(t, np.float32)
    amax = float(np.abs(t).max())
    e = int(np.floor(np.log2(target / max(amax, 1e-30))))
    ts = t * (2.0 ** e)
    hi = ts.astype(E4NP)
    lo = (ts - hi.astype(np.float32)).astype(E4NP)
    return hi, lo, e


def _prep_inputs(x, mask, position_ids, Wq, Wk, Wv, Wo):
    x = np.asarray(x, np.float32)
    Wq = np.asarray(Wq, np.float32)
    Wk = np.asarray(Wk, np.float32)
    Wv = np.asarray(Wv, np.float32)
    Wo = np.asarray(Wo, np.float32)
    block_kind, mixed_vals, exact_binary = _classify_mask(
        np.asarray(mask)[0, 0])
    if mixed_vals is not None and exact_binary:
        mixed_vals = mixed_vals.astype(ml_dtypes.bfloat16)

    # global power-2 scales shared by all cores (compiled constants)
    ax = int(np.floor(np.log2(120.0 / max(float(np.abs(x).max()), 1e-30))))
    sqk = 2.0 ** int(np.floor(np.log2(
        120.0 / max(float(np.abs(Wq).max()), float(np.abs(Wk).max()), 1e-30))))
    bqk = int(np.log2(sqk))
    bv = int(np.floor(np.log2(120.0 / max(float(np.abs(Wv).max()), 1e-30))))
    bo = int(np.floor(np.log2(120.0 / max(float(np.abs(Wo).max()), 1e-30))))
    scales = {"ax": ax, "bqk": bqk, "bv": bv, "bo": bo}

    def enc_fixed(t, e):
        ts = np.asarray(t, np.float32) * (2.0 ** e)
        hi = ts.astype(E4NP)
        lo = (ts - hi.astype(np.float32)).astype(E4NP)
        return np.ascontiguousarray(hi), np.ascontiguousarray(lo)

    desc = 2.0 ** (-(ax + bqk))
    cstab = _rope_tables(np.asarray(position_ids), desc)
    onescol = np.full((P, 1), 2.0 ** (-OC), np.float32)
    ident = np.eye(P, dtype=np.float32)

    in_maps = []
    for core in range(NCORE):
        b, g = divmod(core, KVH)
        xbh, xbl = enc_fixed(x[b].T, ax)
        qh_, ql_ = enc_fixed(Wq[:, g * HPC * DH:(g + 1) * HPC * DH], bqk)
        kh_, kl_ = enc_fixed(Wk[:, g * DH:(g + 1) * DH], bqk)
        vh_, vl_ = enc_fixed(Wv[:, g * DH:(g + 1) * DH], bv)
        oh_, ol_ = enc_fixed(Wo[g * HPC * DH:(g + 1) * HPC * DH, :], bo)
        m = {
            "xh": xbh, "xl": xbl,
            "wqh": qh_, "wql": ql_,
            "wkh": kh_, "wkl": kl_,
            "wvh": vh_, "wvl": vl_,
            "woh": oh_, "wol": ol_,
            "cstab": cstab,
            "onescol": onescol, "ident": ident,
        }
        if mixed_vals is not None:
            m["mmask"] = mixed_vals
        in_maps.append(m)
    return (block_kind, scales, exact_binary), in_maps


def kernel(x, mask, position_ids, Wq, Wk, Wv, Wo):
    (block_kind, scales, exact_binary), in_maps = _prep_inputs(
        x, mask, position_ids, Wq, Wk, Wv, Wo)
    nc = build_program(block_kind, scales, mask_exact_binary=exact_binary)
    res = run_bass_kernel_spmd(nc, in_maps, core_ids=list(range(NCORE)))
    out = np.zeros((B, S, DIM), np.float32)
    for core in range(NCORE):
        b = core // KVH
        out[b] += res.results[core]["y"].astype(np.float32)
    return out
